# revision 14
# baseline (speedup 1.0000x reference)
"""Trainium2 Bass kernel for nn_MACAM (cross-attn modulation + instance norm).

Strategy: pure data parallel - batch B=16 sharded 2 samples per core over 8
NeuronCores.  Per sample the conv + fc_k are folded on the host into a single
matrix Mf = fc_k_w.T @ conv_w so the device computes
    kcT = Mf.T @ ws.T + c0,   attn = kcT.T @ h + kb        (kb host-folded)
The argmax/gather runs on-chip as a one-hot matmul.

v2 schedule: the h load is cut into 1 MB halves on the SWDGE (gpsimd) queue
so chunks land in order s0c0..s1c3; instance-norm bn_stats and the attn
accumulation matmuls chase each half-chunk as it arrives, so sample 0's
modulation-map loop starts as soon as its h finishes loading (~25us) instead
of after the whole load (~48us).  All 8 PSUM banks hold s0's attn pieces
during the load; the map loop then rotates through them.  Sample 1's stats
interleave into s0's map loop (DVE), its attn pieces are computed
just-in-time in its own map loop.  beta2 = beta + t2*gamma is accumulated on
the PE (identity matmul) into the beta-map PSUM; output is written bf16 and
upcast on the host.
"""

import os
import sys

os.environ.setdefault("MYCRO_LOCAL_CACHE", "1")
sys.path.insert(0, "/opt/trn_rl_repo")

import numpy as np

import concourse.bacc as bacc
import concourse.bass as bass
import concourse.mybir as mybir
import concourse.tile as tile
from concourse.bass_utils import run_bass_kernel_spmd

N_CORES = 8
B, C, H, W = 16, 512, 64, 64
HW = H * W
L, D, Q = 64, 512, 512
S = B // N_CORES          # samples per core
EPS = 1e-5
NP = 8                    # HW pieces of 512
NC4 = 4                   # channel chunks of 128

f32 = mybir.dt.float32
f32r = mybir.dt.float32r
bf16 = mybir.dt.bfloat16
AF = mybir.ActivationFunctionType
ALU = mybir.AluOpType
AX = mybir.AxisListType


def _build_program():
    nc = bacc.Bacc("TRN2", target_bir_lowering=False, debug=False,
                   num_devices=N_CORES)
    dt_ = nc.dram_tensor
    h_d = dt_("h", [S, C, H, W], f32, kind="ExternalInput").ap()
    u8 = mybir.dt.uint8
    WB = 16128
    SB = 3584
    wblk_d = dt_("wblk", [128, WB], u8, kind="ExternalInput").ap()
    spk_d = dt_("spk", [S, 128, SB], u8, kind="ExternalInput").ap()
    fb_d = dt_("fc_b_row", [1, 2 * C], f32, kind="ExternalInput").ap()
    ones1_d = dt_("ones1", [1, L], f32, kind="ExternalInput").ap()
    sel8_d = dt_("sel8", [8, 8 * L], f32, kind="ExternalInput").ap()
    out_d = dt_("out", [S, C, HW], bf16, kind="ExternalOutput").ap()

    h_v = h_d.rearrange("s (n p) a b -> s n p (a b)", p=128)     # [S,4,128,4096]
    out_v = out_d.rearrange("s (n p) q -> s n p q", p=128)

    with tile.TileContext(nc) as tc:
        with (
            tc.tile_pool(name="wpool", bufs=1) as wpool,
            tc.tile_pool(name="hpool", bufs=8) as hpool,
            tc.tile_pool(name="attnpool", bufs=2) as attnpool,
            tc.tile_pool(name="spool", bufs=2) as spool,
            tc.tile_pool(name="piece", bufs=5) as piece,
            tc.tile_pool(name="opool", bufs=8) as opool,
            tc.tile_pool(name="ps_bg", bufs=1, space="PSUM") as ps_bg,
            tc.tile_pool(name="ps_ring", bufs=7, space="PSUM") as ps_ring,
        ):
            # ---- DMA issue order --------------------------------------
            # sync (HWDGE, FIFO): spk0 first (scores deps for s0), then
            # wblk (kcT weights), spk1, tiny rows.
            spks = []
            for s in range(S):
                spk = spool.tile([128, SB], mybir.dt.uint8, tag="spk",
                                 name=f"spk{s}")
                spks.append(spk)
            nc.sync.dma_start(spks[0][:], spk_d[0])
            wblk = wpool.tile([128, WB], mybir.dt.uint8, tag="wblk")
            nc.sync.dma_start(wblk[:], wblk_d)
            nc.sync.dma_start(spks[1][:], spk_d[1])
            fcb_row = wpool.tile([1, 2 * C], f32r, tag="fcb")
            nc.sync.dma_start(fcb_row[:], fb_d.bitcast(f32r))
            ones1r = wpool.tile([1, L], f32r, tag="ones1r")
            nc.sync.dma_start(ones1r[:], ones1_d.bitcast(f32r))
            sel8 = wpool.tile([8, 8 * L], f32r, tag="sel8")
            nc.sync.dma_start(sel8[:], sel8_d.bitcast(f32r))

            # gpsimd (SWDGE, FIFO): h in 1MB halves, s0 first.
            hts = []
            for s in range(S):
                h_t = []
                for cc in range(NC4):
                    t = hpool.tile([128, HW], bf16, tag="h", name=f"h{s}{cc}")
                    for hf in range(2):
                        nc.gpsimd.dma_start(
                            t[:, hf * 2048:(hf + 1) * 2048],
                            h_v[s, cc][:, hf * 2048:(hf + 1) * 2048])
                    h_t.append(t)
                hts.append(h_t)

            # ---- packed-weight views ----------------------------------
            mf_t = [wblk[:, j * 1024:(j + 1) * 1024].bitcast(bf16)
                    for j in range(4)]
            fw_t = [wblk[:, 4096 + j * 2048:4096 + (j + 1) * 2048].bitcast(bf16)
                    for j in range(4)]
            ones64 = wblk[0:L, 14336:14592].bitcast(f32)
            nid64 = wblk[0:L, 14592:14848].bitcast(f32)
            id64b = wblk[0:L, 14848:14976].bitcast(bf16)
            id128b = wblk[:, 15232:15488].bitcast(bf16)
            id128 = wblk[:, 15488:16000].bitcast(f32)
            tinyb = wblk[:, 16000:16064].bitcast(f32)
            c0_col = tinyb[:, 0:4]
            inw_col = tinyb[:, 4:8]
            inb_col = tinyb[:, 8:12]
            epz = tinyb[:, 12:14]
            kbs = [tinyb[0:L, 14:15], tinyb[0:L, 15:16]]
            wsT4s, wsT4bs, wtT4s, wts = [], [], [], []
            for s in range(S):
                spk = spks[s]
                wsT4s.append(spk[:, 0:1024].bitcast(f32))
                wtT4s.append(spk[:, 1024:2048].bitcast(f32))
                wsT4bs.append(spk[:, 2048:2560].bitcast(bf16))
                wts.append(spk[0:L, 2560:3584].bitcast(bf16))

            st = [dict() for _ in range(S)]

            def prologue_scores(s):
                wsT4, wtT4, wt_sb = wsT4s[s], wtT4s[s], wts[s]
                # scratch PSUM bank layout (scr):
                #   [0:64, 0:64]    scores       [0:1, 64:128] colsum
                #   [0:64,128:192]  left         [0:64,192:224] PT (bf16)
                #   [0:8, 224:352]  stT          [0:128,384:448] kcT (4x)
                #   [0:128,448:512] waT (4x)
                scr = ps_bg.tile([128, 512], f32, tag="bg")
                st[s]["scr"] = scr

                scores_ps = scr[0:L, 0:L]
                for j in range(4):
                    nc.tensor.matmul(
                        scores_ps,
                        wsT4[:, j * L:(j + 1) * L], wtT4[:, j * L:(j + 1) * L],
                        start=(j == 0), stop=(j == 3))
                scores_sb = spool.tile([L, L], f32, tag="scores_sb")
                nc.scalar.copy(scores_sb[:], scores_ps)
                colsum_ps = scr[0:1, 64:64 + L]
                nc.tensor.matmul(colsum_ps, ones64[:, 0:1], scores_sb[:],
                                 start=True, stop=True)
                colsum_row = spool.tile([1, L], f32, tag="colsum")
                nc.scalar.copy(colsum_row[:], colsum_ps)
                left_ps = scr[0:L, 128:128 + L]
                nc.tensor.matmul(left_ps, ones64[0:1, :], colsum_row[:],
                                 start=True, stop=False)
                nc.tensor.matmul(left_ps, nid64[:], scores_sb[:],
                                 start=False, stop=True)
                rowmax = spool.tile([L, 1], f32, tag="rowmax")
                nc.vector.tensor_reduce(rowmax[:], left_ps, AX.X, ALU.max)
                P_sb = spool.tile([L, L], bf16, tag="P_sb")
                nc.vector.tensor_scalar(P_sb[:], left_ps, rowmax[:], None,
                                        ALU.is_equal)
                PT_ps = scr[0:L, 192:224].bitcast(bf16)
                nc.tensor.transpose(PT_ps, P_sb[:], id64b[:])
                PT_sb = spool.tile([L, L], bf16, tag="PT_sb")
                nc.scalar.copy(PT_sb[:], PT_ps)

                # kcT = Mf.T @ ws.T + c0
                kcT_sb = spool.tile([128, L * NC4], bf16, tag="kcT_sb")
                st[s]["kcT"] = kcT_sb
                wsT4b = wsT4bs[s]
                for cc in range(NC4):
                    kcT_ps = scr[0:128, 384:384 + L]
                    for j in range(4):
                        nc.tensor.matmul(
                            kcT_ps, mf_t[j][:, cc * 128:(cc + 1) * 128],
                            wsT4b[:, j * L:(j + 1) * L],
                            start=(j == 0), stop=(j == 3))
                    nc.scalar.activation(
                        kcT_sb[:, cc * L:(cc + 1) * L],
                        kcT_ps, AF.Identity, bias=c0_col[:, cc:cc + 1])

                # w_allocT
                waT_sb = spool.tile([128, 4 * L], bf16, tag="waT_sb")
                st[s]["waT"] = waT_sb
                for j in range(4):
                    waT_ps = scr[0:128, 448:448 + L]
                    nc.tensor.matmul(waT_ps, wt_sb[:, j * 128:(j + 1) * 128],
                                     PT_sb[:], start=True, stop=True)
                    nc.scalar.copy(waT_sb[:, j * L:(j + 1) * L], waT_ps)

            def stats_slice(s, cc, k, width=512):
                # one bn_stats over h[s][cc][:, k*width:(k+1)*width]
                if "st_col" not in st[s]:
                    st[s]["st_col"] = spool.tile([128, 8], f32, tag="st_col",
                                                 name=f"st_col{s}")
                    st[s]["st6"] = {}
                    st[s]["nst"] = {}
                if cc not in st[s]["st6"]:
                    n = HW // width
                    st[s]["st6"][cc] = spool.tile([128, 6 * n], f32, tag="st6",
                                                  name=f"st6_{s}_{cc}")
                    st[s]["nst"][cc] = n
                st6 = st[s]["st6"][cc]
                nc.vector.bn_stats(
                    st6[:, k * 6:(k + 1) * 6],
                    hts[s][cc][:, k * width:(k + 1) * width])

            def stats_aggr(s, cc):
                st_col, st6 = st[s]["st_col"], st[s]["st6"][cc]
                mv = spool.tile([128, 2], f32, tag="mv")
                nc.vector.bn_aggr(mv[:], st6[:])
                sd = spool.tile([128, 1], f32, tag="sd")
                nc.scalar.activation(sd[:], mv[:, 1:2], AF.Sqrt, bias=epz[:, 0:1])
                rs = spool.tile([128, 1], f32, tag="rs")
                nc.vector.reciprocal(rs[:], sd[:])
                nc.vector.tensor_tensor(
                    st_col[:, cc:cc + 1], rs[:], inw_col[:, cc:cc + 1],
                    ALU.mult)
                ms = spool.tile([128, 1], f32, tag="ms")
                nc.vector.tensor_tensor(ms[:], mv[:, 0:1],
                                        st_col[:, cc:cc + 1], ALU.mult)
                nc.vector.tensor_tensor(st_col[:, 4 + cc:5 + cc],
                                        inb_col[:, cc:cc + 1], ms[:],
                                        ALU.subtract)

            def prologue_fold(s):
                scr, st_col, waT_sb = st[s]["scr"], st[s]["st_col"], st[s]["waT"]
                # t2 broadcast to [64,512]
                stT_ps = scr[0:8, 224:352]
                nc.tensor.transpose(stT_ps, st_col[:], id128[:])
                st8r = spool.tile([8, 128], f32r, tag="st8r")
                nc.scalar.copy(st8r[:], stT_ps)
                t2m_t = ps_ring.tile([128, 512], f32, tag="ring")
                t2m_ps = t2m_t[0:L, :]
                for j in range(4):
                    nc.tensor.matmul(t2m_ps[:, j * 128:(j + 1) * 128],
                                     sel8[:, (4 + j) * L:(5 + j) * L], st8r[:],
                                     start=True, stop=True)
                t2m_sb = spool.tile([L, C], bf16, tag="t2m_sb")
                nc.scalar.copy(t2m_sb[:], t2m_ps)
                smap_t = ps_ring.tile([128, 512], f32, tag="ring")
                smap_ps = smap_t[0:L, :]
                for j in range(4):
                    nc.tensor.matmul(smap_ps[:, j * 128:(j + 1) * 128],
                                     sel8[:, j * L:(j + 1) * L], st8r[:],
                                     start=True, stop=True)
                smap_sb = spool.tile([L, C], bf16, tag="smap_sb")
                nc.scalar.copy(smap_sb[:], smap_ps)

                # gamma then beta, sequentially through scr[0:64, :]
                for j in range(4):
                    nc.tensor.matmul(
                        scr[0:L, :], waT_sb[:, j * L:(j + 1) * L],
                        fw_t[j][:, C:2 * C], start=(j == 0), stop=False)
                nc.tensor.matmul(scr[0:L, :], ones1r[:], fcb_row[:, C:2 * C],
                                 start=False, stop=True)
                # gamma2 = s * gamma ;  tg = t2 * gamma  (both from PSUM)
                gbg = spool.tile([L, C], bf16, tag="gbg")
                nc.vector.tensor_tensor(gbg[:], scr[0:L, :], smap_sb[:],
                                        ALU.mult)
                st[s]["gbg"] = gbg
                nc.vector.tensor_tensor(t2m_sb[:], scr[0:L, :], t2m_sb[:],
                                        ALU.mult)
                for j in range(4):
                    nc.tensor.matmul(
                        scr[0:L, :], waT_sb[:, j * L:(j + 1) * L],
                        fw_t[j][:, 0:C], start=(j == 0), stop=False)
                nc.tensor.matmul(scr[0:L, :], ones1r[:], fcb_row[:, 0:C],
                                 start=False, stop=True)
                # beta2 = beta + t2*gamma
                gbb = spool.tile([L, C], bf16, tag="gbb")   # beta2
                nc.vector.tensor_tensor(gbb[:], scr[0:L, :], t2m_sb[:], ALU.add)
                st[s]["gbb"] = gbb
                if "attn_sb" not in st[s]:
                    st[s]["attn_sb"] = attnpool.tile(
                        [L, HW], bf16, tag="attn_sb", name=f"attn_sb{s}")
                st[s]["pend"] = []

            # ---- s0: chunk-paced attn accumulation during the load ----
            def attn_acc_start(s):
                # 7 persistent [64,512] accumulators on the ring (piece 7 is
                # computed just-in-time once all chunks are in; the scr bank
                # stays free for the other sample's small path).
                acc = [ps_ring.tile([128, 512], f32, tag="ring",
                                    name=f"acc{s}{p}") for p in range(7)]
                st[s]["acc"] = acc

            def attn_acc_chunk(s, cc, hf):
                # pieces hf*4..hf*4+3 get their cc-th K-chunk contribution
                kcT_sb, h_t, acc = st[s]["kcT"], hts[s], st[s]["acc"]
                for pp in range(hf * 4, min(hf * 4 + 4, 7)):
                    nc.tensor.matmul(
                        acc[pp][0:L, :], kcT_sb[:, cc * L:(cc + 1) * L],
                        h_t[cc][:, pp * 512:(pp + 1) * 512],
                        start=(cc == 0), stop=(cc == 3))

            def attn_acc_finalize(s, hf):
                if "attn_sb" not in st[s]:
                    st[s]["attn_sb"] = attnpool.tile(
                        [L, HW], bf16, tag="attn_sb", name=f"attn_sb{s}")
                attn_sb, acc = st[s]["attn_sb"], st[s]["acc"]
                if hf == 1:
                    # piece 7 just-in-time (all chunks present by now); the
                    # ring rotation reuses piece 0's bank after its copy-out
                    kcT_sb, h_t = st[s]["kcT"], hts[s]
                    p7 = ps_ring.tile([128, 512], f32, tag="ring")
                    for cc in range(NC4):
                        nc.tensor.matmul(
                            p7[0:L, :], kcT_sb[:, cc * L:(cc + 1) * L],
                            h_t[cc][:, 7 * 512:8 * 512],
                            start=(cc == 0), stop=(cc == 3))
                    acc.append(p7)
                for pp in range(hf * 4, hf * 4 + 4):
                    nc.scalar.activation(
                        attn_sb[:, pp * 512:(pp + 1) * 512],
                        acc[pp][0:L, :], AF.Identity, bias=kbs[s][:])

            def do_attn(s, pp):
                # just-in-time attn for one piece (used for s1)
                kcT_sb, h_t, attn_sb = st[s]["kcT"], hts[s], st[s]["attn_sb"]
                attn_t = ps_ring.tile([128, 512], f32, tag="ring")
                attn_ps = attn_t[0:L, :]
                for cc in range(NC4):
                    nc.tensor.matmul(
                        attn_ps, kcT_sb[:, cc * L:(cc + 1) * L],
                        h_t[cc][:, pp * 512:(pp + 1) * 512],
                        start=(cc == 0), stop=(cc == 3))
                nc.scalar.activation(attn_sb[:, pp * 512:(pp + 1) * 512],
                                     attn_ps, AF.Identity, bias=kbs[s][:])

            # output staging: [128, 1024] bf16 tiles, one per cc, covering
            # two consecutive pieces -> out DMA per (cc, piece-pair) halves
            # the sync-engine DMA-issue count.
            ostage = [dict() for _ in range(S)]

            def finish_piece(s, cc, pp, bm_ps, tmp, dve_fin):
                if pp % 2 == 0:
                    ostage[s][cc] = opool.tile([128, 1024], bf16, tag="outp",
                                               name=f"op{s}{cc}{pp}")
                o2 = ostage[s][cc]
                half = o2[:, (pp % 2) * 512:(pp % 2) * 512 + 512]
                if dve_fin:
                    # out = bm_ps + tmp in one DVE op (beta matmul ended its
                    # accumulation group already)
                    nc.vector.scalar_tensor_tensor(
                        half, bm_ps[:], 1.0, tmp[:], ALU.mult, ALU.add)
                else:
                    nc.tensor.matmul(bm_ps[:], id128b[:], tmp[:],
                                     start=False, stop=True)
                    nc.scalar.copy(half, bm_ps[:])
                if pp % 2 == 1:
                    nc.sync.dma_start(
                        out_v[s, cc][:, (pp - 1) * 512:(pp + 1) * 512], o2[:])

            def do_maps(s, pp, side=None, gp_cc=(), dve_fin_cc=()):
                attn_sb, gbg, gbb = st[s]["attn_sb"], st[s]["gbg"], st[s]["gbb"]
                h_t, pend = hts[s], st[s]["pend"]
                aps = attn_sb[:, pp * 512:(pp + 1) * 512]
                # phase 1: all gamma-map matmuls + multiplies, so the
                # multiplies overlap the beta-map matmuls of phase 2
                tmps = []
                for cc in range(NC4):
                    gm_ps = ps_ring.tile([128, 512], f32, tag="ring")
                    nc.tensor.matmul(
                        gm_ps[:], gbg[:, cc * 128:(cc + 1) * 128],
                        aps, start=True, stop=True)
                    tmp = piece.tile([128, 512], bf16, tag="tmp")
                    if cc in gp_cc:
                        gmc = piece.tile([128, 512], bf16, tag="gmc")
                        nc.scalar.copy(gmc[:], gm_ps[:])
                        nc.gpsimd.tensor_tensor(
                            tmp[:], gmc[:],
                            h_t[cc][:, pp * 512:(pp + 1) * 512], ALU.mult)
                    else:
                        nc.vector.tensor_tensor(
                            tmp[:], gm_ps[:],
                            h_t[cc][:, pp * 512:(pp + 1) * 512], ALU.mult)
                    tmps.append(tmp)
                    if side:
                        side.pop(0)()
                    if side:
                        side.pop(0)()
                # phase 2: beta-map matmuls + accumulate + copy out
                for cc in range(NC4):
                    dve_fin = cc in dve_fin_cc
                    bm_ps = ps_ring.tile([128, 512], f32, tag="ring")
                    nc.tensor.matmul(
                        bm_ps[:], gbb[:, cc * 128:(cc + 1) * 128],
                        aps, start=True, stop=dve_fin)
                    pend.append((s, cc, pp, bm_ps, tmps[cc], dve_fin))
                    if len(pend) > st[s].get("pend_depth", 1):
                        finish_piece(*pend.pop(0))

            def stats_thunks(s, width=512):
                th = []
                n = HW // width
                for cc in range(NC4):
                    for k in range(n):
                        th.append(lambda s=s, cc=cc, k=k: stats_slice(
                            s, cc, k, width))
                    th.append(lambda s=s, cc=cc: stats_aggr(s, cc))
                return th

            # ================= emission schedule =======================
            # s0 small path first (deps: spk0 only / + wblk for kcT)
            prologue_scores(0)
            attn_acc_start(0)
            # chunk-paced: stats + attn accumulation follow the h halves
            for cc in range(NC4):
                for hf in range(2):
                    stats_slice(0, cc, 4 * hf)
                    stats_slice(0, cc, 4 * hf + 1)
                    attn_acc_chunk(0, cc, hf)
                    stats_slice(0, cc, 4 * hf + 2)
                    stats_slice(0, cc, 4 * hf + 3)
                stats_aggr(0, cc)
                if cc == 1:
                    # s1's small path rides the load window (PE mostly idle;
                    # spk1 has landed by now)
                    prologue_scores(1)
            attn_acc_finalize(0, 0)
            attn_acc_finalize(0, 1)
            prologue_fold(0)

            # s0 maps; s1 stats ride along on DVE, paced to s1's h arrival
            # (~3 thunks per piece from piece 2 on; popping too early would
            # stall the in-order DVE stream on the h DMA semaphore).
            thunks = stats_thunks(1)             # 36 thunks
            side_sched = [0, 0, 5, 6, 6, 6, 7, 6]
            st[0]["pend_depth"] = 2
            for pp in range(NP):
                n = side_sched[pp]
                side = [thunks.pop(0) for _ in range(min(n, len(thunks)))]
                side += [lambda: None] * (8 - len(side))
                do_maps(0, pp, side, gp_cc=(1, 2, 3), dve_fin_cc=())
            while st[0]["pend"]:
                finish_piece(*st[0]["pend"].pop(0))
            while thunks:
                thunks.pop(0)()

            # s1: attn prefetched 4 pieces ahead of maps (baseline-proven
            # W1 shape: id-accum on PE, one gpsimd mult per piece)
            st[1]["attn_sb"] = attnpool.tile([L, HW], bf16, tag="attn_sb",
                                             name="attn_sb1")
            for pp in range(4):
                do_attn(1, pp)
            prologue_fold(1)
            for pp in range(4, NP):
                do_maps(1, pp - 4, gp_cc=(3,), dve_fin_cc=())
                do_attn(1, pp)
            for pp in range(NP - 4, NP):
                do_maps(1, pp, gp_cc=(3,), dve_fin_cc=())
            while st[1]["pend"]:
                finish_piece(*st[1]["pend"].pop(0))

    nc.compile()
    return nc


_NC_CACHE = None


def _get_nc():
    global _NC_CACHE
    if _NC_CACHE is None:
        _NC_CACHE = _build_program()
    return _NC_CACHE


def make_in_maps(inputs):
    import ml_dtypes
    f8 = np.float64
    bfd = ml_dtypes.bfloat16
    h = np.ascontiguousarray(inputs["h"], dtype=np.float32)
    ws = np.asarray(inputs["w_source"], dtype=np.float32)
    wt = np.asarray(inputs["w_target"], dtype=np.float32)
    conv_w = np.asarray(inputs["conv_w"], dtype=np.float32)
    conv_b = np.asarray(inputs["conv_b"], dtype=np.float32)
    fc_k_w = np.asarray(inputs["fc_k_w"], dtype=np.float32)
    fc_k_b = np.asarray(inputs["fc_k_b"], dtype=np.float32)
    fc_w = np.asarray(inputs["fc_w"], dtype=np.float32)
    fc_b = np.asarray(inputs["fc_b"], dtype=np.float32)
    in_w = np.asarray(inputs["in_w"], dtype=np.float32)
    in_b = np.asarray(inputs["in_b"], dtype=np.float32)

    ws_t = ws.transpose(0, 2, 1)                            # [B, D, L]
    ws_t4 = np.ascontiguousarray(
        ws_t.reshape(B, 4, 128, L).transpose(0, 2, 1, 3).reshape(B, 128, 4 * L))
    ws_t4_bf = np.ascontiguousarray(ws_t4.astype(bfd))
    wt_t4 = np.ascontiguousarray(
        wt.transpose(0, 2, 1).reshape(B, 4, 128, L)
        .transpose(0, 2, 1, 3).reshape(B, 128, 4 * L))
    wt_bf = np.ascontiguousarray(wt.astype(bfd))

    # host folds: Mf = fc_k_w.T @ conv_w ; c0 = conv_w.T @ fc_k_b ;
    # kb[b,l] = ws[b] @ (fc_k_w.T @ conv_b) + fc_k_b . conv_b
    cw2 = conv_w[:, :, 0, 0].astype(f8)                     # [Q, C]
    mf = (fc_k_w.astype(f8).T @ cw2).astype(np.float32)     # [D, C]
    c0 = (cw2.T @ fc_k_b.astype(f8)).astype(np.float32)     # [C]
    vb = fc_k_w.astype(f8).T @ conv_b.astype(f8)            # [D]
    kb = (ws.astype(f8) @ vb
          + fc_k_b.astype(f8) @ conv_b.astype(f8)).astype(np.float32)  # [B,L]

    # ---- packed weight block [128, 16128] bytes ----
    wblk = np.zeros((128, 16128), dtype=np.uint8)

    def put(col, arr, rows=128):
        b = np.ascontiguousarray(arr).view(np.uint8)
        b = b.reshape(rows, -1)
        wblk[:rows, col:col + b.shape[1]] = b
        return col + b.shape[1]

    mfb = mf.astype(bfd)
    for j in range(4):
        put(j * 1024, mfb[j * 128:(j + 1) * 128, :])
    fwb = fc_w.T.astype(bfd)
    for j in range(4):
        put(4096 + j * 2048, fwb[j * 128:(j + 1) * 128, :])
    put(14336, np.ones((L, L), dtype=np.float32), rows=L)
    put(14592, -np.eye(L, dtype=np.float32), rows=L)
    put(14848, np.eye(L, dtype=bfd), rows=L)
    put(15232, np.eye(128, dtype=bfd))
    put(15488, np.eye(128, dtype=np.float32))
    tiny = np.zeros((128, 16), dtype=np.float32)
    tiny[:, 0:4] = c0.reshape(4, 128).T
    tiny[:, 4:8] = in_w.reshape(4, 128).T
    tiny[:, 8:12] = in_b.reshape(4, 128).T
    tiny[:, 12] = EPS

    shared = {
        "fc_b_row": np.ascontiguousarray(fc_b.reshape(1, 2 * C)),
        "ones1": np.ones((1, L), dtype=np.float32),
        "sel8": np.repeat(np.eye(8, dtype=np.float32), L, axis=1),
    }
    in_maps = []
    for i in range(N_CORES):
        lo = i * S
        wb = wblk.copy()
        t = tiny.copy()
        t[0:64, 14] = kb[lo]
        t[0:64, 15] = kb[lo + 1]
        wb[:, 16000:16064] = t.view(np.uint8).reshape(128, 64)
        spk = np.zeros((S, 128, 3584), dtype=np.uint8)
        for s in range(S):
            b = lo + s
            spk[s, :, 0:1024] = ws_t4[b].view(np.uint8).reshape(128, 1024)
            spk[s, :, 1024:2048] = wt_t4[b].view(np.uint8).reshape(128, 1024)
            spk[s, :, 2048:2560] = ws_t4_bf[b].view(np.uint8).reshape(128, 512)
            spk[s, 0:64, 2560:3584] = wt_bf[b].view(np.uint8).reshape(64, 1024)
        in_maps.append({
            "h": h[lo:lo + S],
            "wblk": wb,
            "spk": spk,
            **shared,
        })
    return in_maps


def kernel(**inputs):
    in_maps = make_in_maps(inputs)
    nc = _get_nc()
    res = run_bass_kernel_spmd(nc, in_maps, core_ids=list(range(N_CORES)))
    out = np.concatenate(
        [np.asarray(res.results[i]["out"]) for i in range(N_CORES)], axis=0)
    return out.astype(np.float32).reshape(B, C, H, W)


if __name__ == "__main__":
    rng = np.random.default_rng(0)
    ins = {
        "h": rng.standard_normal((B, C, H, W), dtype=np.float32),
        "w_source": rng.standard_normal((B, L, D), dtype=np.float32),
        "w_target": rng.standard_normal((B, L, D), dtype=np.float32),
        "conv_w": (rng.standard_normal((Q, C, 1, 1), dtype=np.float32)
                   / np.sqrt(C)),
        "conv_b": np.zeros(Q, np.float32),
        "fc_k_w": (rng.standard_normal((Q, D), dtype=np.float32)
                   / np.sqrt(D)),
        "fc_k_b": np.zeros(Q, np.float32),
        "fc_w": (rng.standard_normal((2 * C, D), dtype=np.float32)
                 / np.sqrt(D)),
        "fc_b": np.zeros(2 * C, np.float32),
        "in_w": np.ones(C, np.float32),
        "in_b": np.zeros(C, np.float32),
    }
    out = kernel(**ins)
    print("out", out.shape, out.dtype, float(np.abs(out).max()))
